# revision 2
# baseline (speedup 1.0000x reference)
"""Trainium2 Bass kernel for nn_KernelMachine (random Fourier features).

out[n,m] = sum_f sqrt(2/F) * cos(x_n . a_f + b_f) * W[f*M+m]

Data-parallel over 8 NeuronCores (N sharded, a/b/W replicated).

v4: magic-rounding range reduction fused INTO the PE accumulation chain
(no DVE round pass, no PE corr pass).  Mechanism, validated by probes:
the 128x128 PE array accumulates each output column sequentially (fp32
round per cell) within each 32-row strip, and merges strips pairwise.
A single-strip stationary chain [-prods; +MAGIC; -MAGIC] therefore
computes fl(-t + M) = M - rint(t) at the +M cell (magic rounding) and
-rint(t) at the -M cell (exact Sterbenz), leaving the small integer
-rint(t_1limb) in PSUM.  A second matmul PSUM-accumulates the 2-limb
value +t, so PSUM ends up with s = t - rint(t1) in [-0.503, 0.503],
and ACT applies Sin(2*pi*s) directly (the Sin spline is accurate to
|x| <= 3.3 rad, measured 8e-8 at 3.16 — the old [-pi,pi] assumption was
too conservative).

Per core (N_loc=4096, D=16, F=4096, M=16), per tile (f-chunk of 128 x
n-group of 1024, 128 tiles):
  mr (PE):  2 concurrent single-strip matmuls (K=20, tile_position
            (0,0)/(64,0), 512 cols each) -> P = -rint(t_1limb).
  mt (PE):  2 concurrent half-array matmuls (K=38: [ah;0x4;ah;bh;bl]
            vs shared moving [xh;1x4;xl;1,1]) accumulate +t_2limb ->
            P = s.  start/stop pattern as one accumulation group.
  sin(ACT): phi = Sin(2*pi*s) -> SBUF bf16.
  m2 (PE):  cps[J][32g:32g+32] += wsc[:,c,:].T @ phi-half, col-group
            g = (c + 2h) % 4 per n-half h; wsc zero-padded to M=32.
  epilogue: ACT copies cps -> SBUF bf16; PE transpose+4-way reduce via
            SEL selector matmuls; DVE scales by 1/W_PRESCALE; DMA out.

PE streaming/tile: 512 (mr pair) + 512 (mt pair) + 1024 (m2) = 2048 cyc
(vs 3072 baseline); DVE idle except epilogue; ACT (~1.0us/tile) is the
expected wall.
"""

import math

import numpy as np

import concourse.bass as bass
import concourse.tile as tile
from concourse import bacc, mybir
from concourse.bass_utils import run_bass_kernel_spmd

F32 = mybir.dt.float32
BF16 = mybir.dt.bfloat16

N, D, F, M = 32768, 16, 4096, 16
NCORES = 8
NLOC = N // NCORES            # 4096 rows per core
FC = F // 128                 # 32 f-chunks of 128
NJ = NLOC // 1024             # 4 n-groups of 1024

MAGIC = float(np.float32(1.5 * 2 ** 23))
TWO_PI = float(2.0 * np.pi)
W_PRESCALE = 256.0            # keep wsc bf16 away from subnormals

M2_LAG = 6                    # m2 consumes phi 6 iterations behind mr/mt
NT = FC * NJ                  # 128 tiles

_CACHE = {}


def build_nc():
    nc = bacc.Bacc(None, target_bir_lowering=False)

    # compact inputs: xq rows 0:38 = [xh(16); 1(4); xl(16); 1; 1]
    # aq rows 0:20 = mr stationary [-ah; -bh; -bl; +M; -M]
    #    rows 20:58 = mt stationary [ah; 0,0,0,0; ah; bh; bl]
    xq_in = nc.dram_tensor("xq_in", [38, NLOC], BF16, kind="ExternalInput")
    aq_in = nc.dram_tensor("aq_in", [58, FC, 128], BF16, kind="ExternalInput")
    wsc_in = nc.dram_tensor("wsc_in", [128, FC, 2 * M], BF16, kind="ExternalInput")
    sel_in = nc.dram_tensor("sel_in", [112, 16], BF16, kind="ExternalInput")
    out_t = nc.dram_tensor("out", [NLOC, M], F32, kind="ExternalOutput")

    with tile.TileContext(nc) as tc:
        with (
            tc.tile_pool(name="const", bufs=1) as const,
            tc.tile_pool(name="php", bufs=12) as php,
            tc.tile_pool(name="sg", bufs=3) as sg,
            tc.tile_pool(name="ob", bufs=3) as ob,
            tc.tile_pool(name="pst", bufs=3, space="PSUM") as pst,
            tc.tile_pool(name="pcs", bufs=1, space="PSUM") as pcs,
        ):
            # ---------------- constants ----------------
            # DMA order: first-needed first.
            xq = const.tile([128, NLOC], BF16, tag="xq")
            aqr = const.tile([128, FC, 128], BF16, tag="aqr")
            aqt = const.tile([128, FC, 128], BF16, tag="aqt")
            wsc = const.tile([128, FC, 2 * M], BF16, tag="wsc")
            sel = const.tile([112, 16], BF16, tag="sel")

            def dma_x(cols):
                nc.sync.dma_start(out=xq[0:38, cols], in_=xq_in[0:38, cols])
                nc.sync.dma_start(out=xq[64:102, cols], in_=xq_in[0:38, cols])

            def dma_ar(chunks):
                nc.sync.dma_start(out=aqr[0:20, chunks, :], in_=aq_in[0:20, chunks, :])
                nc.sync.dma_start(out=aqr[64:84, chunks, :], in_=aq_in[0:20, chunks, :])

            def dma_at(chunks):
                nc.sync.dma_start(out=aqt[0:38, chunks, :], in_=aq_in[20:58, chunks, :])
                nc.sync.dma_start(out=aqt[64:102, chunks, :], in_=aq_in[20:58, chunks, :])

            dma_x(slice(0, 1024))
            dma_ar(slice(0, 1))
            dma_at(slice(0, 1))
            dma_x(slice(1024, 2048))
            dma_ar(slice(1, 4))
            dma_at(slice(1, 4))
            nc.sync.dma_start(out=wsc, in_=wsc_in[:])
            nc.sync.dma_start(out=sel, in_=sel_in[:])
            for p in range(4, FC, 4):
                dma_ar(slice(p, p + 4))
                dma_at(slice(p, p + 4))
            for j in range(2, NJ):
                dma_x(slice(1024 * j, 1024 * (j + 1)))

            # Preload the Sin ACT table during the DMA wait.
            dummy = const.tile([1, 8], F32, tag="dummy")
            nc.gpsimd.memset(dummy, 0.25)
            dummy2 = const.tile([1, 8], BF16, tag="dummy2")
            nc.scalar.activation(out=dummy2, in_=dummy,
                                 func=mybir.ActivationFunctionType.Sin,
                                 bias=0.0, scale=1.0)

            # ---------------- main loop (software-pipelined) ----------------
            t_tiles = {}
            phi_tiles = {}
            cps_by_j = {}

            def emit_epilogue(j):
                cps = cps_by_j.pop(j)
                stage = sg.tile([112, 1024], BF16, tag="stage")
                nc.scalar.copy(out=stage, in_=cps[0:112, :])
                ps2 = pcs.tile([128, 1024], F32, tag="cps")
                for qq in range(8):
                    nc.tensor.matmul(
                        ps2[:, 16 * qq:16 * (qq + 1)],
                        stage[:, 128 * qq:128 * (qq + 1)],
                        sel,
                        start=True, stop=True,
                    )
                obuf = ob.tile([128, 128], F32, tag="obuf")
                for half in range(2):
                    nc.vector.tensor_scalar(
                        out=obuf[:, 64 * half:64 * (half + 1)],
                        in0=ps2[:, 64 * half:64 * (half + 1)],
                        scalar1=1.0 / W_PRESCALE, scalar2=None,
                        op0=mybir.AluOpType.mult,
                    )
                    nc.sync.dma_start(
                        out=out_t[1024 * j + 512 * half:
                                  1024 * j + 512 * (half + 1), :].rearrange(
                            "(qq p) m -> p qq m", qq=4
                        ),
                        in_=obuf[:, 64 * half:64 * (half + 1)].rearrange(
                            "p (qq m) -> p qq m", qq=4
                        ),
                    )

            for it in range(NT + M2_LAG + 1):
                # ---- mr(it) + mt(it) ----
                if it < NT:
                    j, c = divmod(it, FC)
                    tp = pst.tile([128, 1024], F32, tag="t")
                    for h in range(2):
                        grp = 64 * h
                        nc.tensor.matmul(
                            tp[:, 512 * h:512 * (h + 1)],
                            aqr[grp:grp + 20, c, :],
                            xq[grp:grp + 20,
                               1024 * j + 512 * h:1024 * j + 512 * (h + 1)],
                            start=True, stop=False,
                            tile_position=(grp, 0),
                        )
                    for h in range(2):
                        grp = 64 * h
                        nc.tensor.matmul(
                            tp[:, 512 * h:512 * (h + 1)],
                            aqt[grp:grp + 38, c, :],
                            xq[grp:grp + 38,
                               1024 * j + 512 * h:1024 * j + 512 * (h + 1)],
                            start=False, stop=True,
                            tile_position=(grp, 0),
                        )
                    t_tiles[it] = tp
                # ---- sin(it-1) ----
                if 0 <= it - 1 < NT:
                    tp = t_tiles.pop(it - 1)
                    phi = php.tile([128, 1024], BF16, tag="phi")
                    nc.scalar.activation(
                        out=phi, in_=tp,
                        func=mybir.ActivationFunctionType.Sin,
                        bias=0.0, scale=TWO_PI,
                    )
                    phi_tiles[it - 1] = phi
                # ---- m2(it-M2_LAG) ----
                if 0 <= it - M2_LAG < NT:
                    it6 = it - M2_LAG
                    j6, c6 = divmod(it6, FC)
                    if c6 == 0:
                        cps_by_j[j6] = pcs.tile([128, 1024], F32, tag="cps", name="cps")
                    phi = phi_tiles.pop(it6)
                    for h in range(2):
                        gh = (c6 + 2 * h) % 4
                        nc.tensor.matmul(
                            cps_by_j[j6][32 * gh:32 * gh + 32,
                                         512 * h:512 * (h + 1)],
                            wsc[:, c6, :],
                            phi[:, 512 * h:512 * (h + 1)],
                            start=(c6 < 4), stop=(c6 >= 28),
                            tile_position=(0, 32 * gh),
                        )
                    if c6 == FC - 1:
                        emit_epilogue(j6)
    nc.finalize()
    return nc


def _host_prep(a, b, W):
    """Precompute replicated bf16 operand packs (float64 for exact splits)."""
    import ml_dtypes
    bf16 = ml_dtypes.bfloat16
    inv2pi = 1.0 / (2.0 * np.pi)
    a64 = np.asarray(a, dtype=np.float64).T * inv2pi          # [16, F]
    b64 = (np.asarray(b, dtype=np.float64) + np.pi / 2.0) * inv2pi  # [F]
    ah = a64.astype(bf16)                                      # single limb
    bh = b64.astype(bf16)
    bl = (b64 - bh.astype(np.float64)).astype(bf16)

    # aq rows 0:20 = mr stationary [-ah; -bh; -bl; +M; -M]
    # aq rows 20:58 = mt stationary [ah; 0*4; ah; bh; bl]
    aq = np.zeros((58, FC, 128), dtype=bf16)
    for c in range(FC):
        sl = slice(128 * c, 128 * (c + 1))
        aq[0:16, c, :] = -ah[:, sl]
        aq[16, c, :] = -bh[sl]
        aq[17, c, :] = -bl[sl]
        aq[18, c, :] = bf16(MAGIC)
        aq[19, c, :] = bf16(-MAGIC)
        aq[20:36, c, :] = ah[:, sl]
        # rows 36:40 exactly zero (match ones rows of xq)
        aq[40:56, c, :] = ah[:, sl]
        aq[56, c, :] = bh[sl]
        aq[57, c, :] = bl[sl]

    scale = math.sqrt(2.0 / F) * W_PRESCALE
    W2 = (np.asarray(W, dtype=np.float64).reshape(F, M) * scale).astype(bf16)
    wsc = np.zeros((128, FC, 2 * M), dtype=bf16)               # zero-padded M
    wsc[:, :, 0:M] = W2.reshape(FC, 128, M).transpose(1, 0, 2)

    sel = np.zeros((112, 16), dtype=bf16)
    for g in range(4):
        for m in range(16):
            sel[32 * g + m, m] = 1.0
    return aq, wsc, sel


def _pack_x(xs):
    """xs [NLOC, D] fp32 -> xq [38, NLOC] bf16: [xh(16); 1(4); xl(16); 1; 1]."""
    import ml_dtypes
    bf16 = ml_dtypes.bfloat16
    x64 = np.asarray(xs, dtype=np.float64).T                   # [16, NLOC]
    xh = x64.astype(bf16)
    xl = (x64 - xh.astype(np.float64)).astype(bf16)
    xq = np.zeros((38, NLOC), dtype=bf16)
    xq[0:16] = xh
    xq[16:20] = bf16(1.0)
    xq[20:36] = xl
    xq[36:38] = bf16(1.0)
    return xq


def make_in_maps(x, a, b, W):
    x = np.ascontiguousarray(np.asarray(x, dtype=np.float32))
    aq, wsc, sel = _host_prep(a, b, W)
    in_maps = []
    for i in range(NCORES):
        in_maps.append({
            "xq_in": _pack_x(x[i * NLOC:(i + 1) * NLOC]),
            "aq_in": aq,
            "wsc_in": wsc,
            "sel_in": sel,
        })
    return in_maps


def kernel(x, a, b, W):
    if "nc" not in _CACHE:
        _CACHE["nc"] = build_nc()
    nc = _CACHE["nc"]
    in_maps = make_in_maps(x, a, b, W)
    res = run_bass_kernel_spmd(nc, in_maps, core_ids=list(range(NCORES)))
    return np.concatenate([r["out"] for r in res.results], axis=0)


# revision 3
# speedup vs baseline: 1.4516x; 1.4516x over previous
"""Trainium2 Bass kernel for nn_KernelMachine (random Fourier features).

out[n,m] = sum_f sqrt(2/F) * cos(x_n . a_f + b_f) * W[f*M+m]

Data-parallel over 8 NeuronCores (N sharded, a/b/W replicated).

v5: the ENTIRE angle computation + range reduction happens inside ONE
matmul per 512-col half.  Mechanism (validated by hardware probes): the
PE accumulates each output column sequentially (fp32 rounding per cell)
within each 32-row strip, then merges strips pairwise in fp32.  With
MAGIC = 1.5*2^23:

  strip0 chain: -xh.ah(16) -bh -bl  -> -t1
                +MAGIC              -> fl(M - t1) = M - rint(t1)
                -MAGIC              -> -rint(t1)   (exact Sterbenz)
                +xl.ah(12 dims)     -> -rint(t1) + u   (small adds)
  strip1 chain: +xh.ah(16) +bh +bl +xl.ah(4 dims) -> t1 + v
  L-node merge: s = t2 - rint(t1),  t2 = full 2-limb angle/2pi

|s| <= 0.5 + |xl.a| ~ 0.51 -> |2*pi*s| <= 3.21 rad, inside the Sin
spline's accurate domain (measured 8e-8 err at |x|<=3.25; the old
[-pi,pi] assumption was too conservative).

Per core (N_loc=4096, D=16, F=4096, M=16), per tile (f-chunk of 128 x
n-group of 1024, 128 tiles):
  m1 (PE):  2 MMs, K=54, tile_position (0,0)/(64,0) -> s in PSUM.
  sin(ACT): phi = Sin(2*pi*s) -> SBUF bf16.
  m2 (PE):  cps[J][32g:32g+32] += wsc[:,c,:].T @ phi-half, col-group
            g = (c + 2h) % 4 per n-half h; wsc zero-padded to M=32.
  epilogue: DVE copies cps -> SBUF bf16; PE transpose+4-way reduce via
            SEL selector matmuls; DVE scales by 1/W_PRESCALE; DMA out.

PE: 4 MMs/tile (2048->1024 streaming cycles vs 3072 baseline).
DVE idle except epilogue.  ACT sin ~1.1us/tile is the expected wall.
"""

import math

import numpy as np

import concourse.bass as bass
import concourse.tile as tile
from concourse import bacc, mybir
from concourse.bass_utils import run_bass_kernel_spmd

F32 = mybir.dt.float32
BF16 = mybir.dt.bfloat16

N, D, F, M = 32768, 16, 4096, 16
NCORES = 8
NLOC = N // NCORES            # 4096 rows per core
FC = F // 128                 # 32 f-chunks of 128
NJ = NLOC // 1024             # 4 n-groups of 1024

MAGIC = float(np.float32(1.5 * 2 ** 23))
TWO_PI = float(2.0 * np.pi)
W_PRESCALE = 256.0            # keep wsc bf16 away from subnormals

M2_LAG = 6                    # m2 consumes phi 6 iterations behind m1
NT = FC * NJ                  # 128 tiles

_CACHE = {}

# xq row layout (54 rows per half-group):
#   0:16  xh      (vs -ah in strip0)
#   16:20 ones    (bh, bl, +M, -M)
#   20:32 xl[0:12]
#   32:48 xh      (vs +ah in strip1)
#   48:50 ones    (bh, bl)
#   50:54 xl[12:16]


def build_nc():
    nc = bacc.Bacc(None, target_bir_lowering=False)

    xq_in = nc.dram_tensor("xq_in", [54, NLOC], BF16, kind="ExternalInput")
    aq_in = nc.dram_tensor("aq_in", [54, FC, 128], BF16, kind="ExternalInput")
    wsc_in = nc.dram_tensor("wsc_in", [128, FC, 2 * M], BF16, kind="ExternalInput")
    sel_in = nc.dram_tensor("sel_in", [112, 16], BF16, kind="ExternalInput")
    out_t = nc.dram_tensor("out", [NLOC, M], F32, kind="ExternalOutput")

    with tile.TileContext(nc) as tc:
        with (
            tc.tile_pool(name="const", bufs=1) as const,
            tc.tile_pool(name="php", bufs=12) as php,
            tc.tile_pool(name="sg", bufs=3) as sg,
            tc.tile_pool(name="ob", bufs=3) as ob,
            tc.tile_pool(name="pst", bufs=3, space="PSUM") as pst,
            tc.tile_pool(name="pcs", bufs=1, space="PSUM") as pcs,
        ):
            # ---------------- constants ----------------
            xq = const.tile([128, NLOC], BF16, tag="xq")
            aq = const.tile([128, FC, 128], BF16, tag="aq")
            wsc = const.tile([128, FC, 2 * M], BF16, tag="wsc")
            sel = const.tile([112, 16], BF16, tag="sel")

            def dma_x(cols):
                nc.sync.dma_start(out=xq[0:54, cols], in_=xq_in[0:54, cols])
                nc.sync.dma_start(out=xq[64:118, cols], in_=xq_in[0:54, cols])

            def dma_a(chunks):
                nc.sync.dma_start(out=aq[0:54, chunks, :], in_=aq_in[0:54, chunks, :])
                nc.sync.dma_start(out=aq[64:118, chunks, :], in_=aq_in[0:54, chunks, :])

            dma_x(slice(0, 1024))
            dma_a(slice(0, 1))
            dma_x(slice(1024, 2048))
            dma_a(slice(1, 4))
            nc.sync.dma_start(out=wsc, in_=wsc_in[:])
            nc.sync.dma_start(out=sel, in_=sel_in[:])
            for p in range(4, FC, 4):
                dma_a(slice(p, p + 4))
            for j in range(2, NJ):
                dma_x(slice(1024 * j, 1024 * (j + 1)))

            # Preload the Sin ACT table during the DMA wait.
            dummy = const.tile([1, 8], F32, tag="dummy")
            nc.gpsimd.memset(dummy, 0.25)
            dummy2 = const.tile([1, 8], BF16, tag="dummy2")
            nc.scalar.activation(out=dummy2, in_=dummy,
                                 func=mybir.ActivationFunctionType.Sin,
                                 bias=0.0, scale=1.0)

            # ---------------- main loop (software-pipelined) ----------------
            t_tiles = {}
            phi_tiles = {}
            cps_by_j = {}

            def emit_epilogue(j):
                cps = cps_by_j.pop(j)
                stage = sg.tile([112, 1024], BF16, tag="stage")
                nc.vector.tensor_copy(out=stage, in_=cps[0:112, :])
                ps2 = pcs.tile([128, 1024], F32, tag="cps")
                for qq in range(8):
                    nc.tensor.matmul(
                        ps2[:, 16 * qq:16 * (qq + 1)],
                        stage[:, 128 * qq:128 * (qq + 1)],
                        sel,
                        start=True, stop=True,
                    )
                obuf = ob.tile([128, 128], F32, tag="obuf")
                for half in range(2):
                    nc.vector.tensor_scalar(
                        out=obuf[:, 64 * half:64 * (half + 1)],
                        in0=ps2[:, 64 * half:64 * (half + 1)],
                        scalar1=1.0 / W_PRESCALE, scalar2=None,
                        op0=mybir.AluOpType.mult,
                    )
                    nc.sync.dma_start(
                        out=out_t[1024 * j + 512 * half:
                                  1024 * j + 512 * (half + 1), :].rearrange(
                            "(qq p) m -> p qq m", qq=4
                        ),
                        in_=obuf[:, 64 * half:64 * (half + 1)].rearrange(
                            "p (qq m) -> p qq m", qq=4
                        ),
                    )

            for it in range(NT + M2_LAG + 1):
                # ---- m1(it) ----
                if it < NT:
                    j, c = divmod(it, FC)
                    tp = pst.tile([128, 1024], F32, tag="t")
                    for h in range(2):
                        grp = 64 * h
                        nc.tensor.matmul(
                            tp[:, 512 * h:512 * (h + 1)],
                            aq[grp:grp + 54, c, :],
                            xq[grp:grp + 54,
                               1024 * j + 512 * h:1024 * j + 512 * (h + 1)],
                            start=True, stop=True,
                            tile_position=(grp, 0),
                        )
                    t_tiles[it] = tp
                # ---- sin(it-1) ----
                if 0 <= it - 1 < NT:
                    tp = t_tiles.pop(it - 1)
                    phi = php.tile([128, 1024], BF16, tag="phi")
                    nc.scalar.activation(
                        out=phi, in_=tp,
                        func=mybir.ActivationFunctionType.Sin,
                        bias=0.0, scale=TWO_PI,
                    )
                    phi_tiles[it - 1] = phi
                # ---- m2(it-M2_LAG) ----
                if 0 <= it - M2_LAG < NT:
                    it6 = it - M2_LAG
                    j6, c6 = divmod(it6, FC)
                    if c6 == 0:
                        cps_by_j[j6] = pcs.tile([128, 1024], F32, tag="cps", name="cps")
                    phi = phi_tiles.pop(it6)
                    for h in range(2):
                        gh = (c6 + 2 * h) % 4
                        nc.tensor.matmul(
                            cps_by_j[j6][32 * gh:32 * gh + 32,
                                         512 * h:512 * (h + 1)],
                            wsc[:, c6, :],
                            phi[:, 512 * h:512 * (h + 1)],
                            start=(c6 < 4), stop=(c6 >= 28),
                            tile_position=(0, 32 * gh),
                        )
                    if c6 == FC - 1:
                        emit_epilogue(j6)
    nc.finalize()
    return nc


def _host_prep(a, b, W):
    """Precompute replicated bf16 operand packs (float64 for exact splits)."""
    import ml_dtypes
    bf16 = ml_dtypes.bfloat16
    inv2pi = 1.0 / (2.0 * np.pi)
    a64 = np.asarray(a, dtype=np.float64).T * inv2pi          # [16, F]
    b64 = (np.asarray(b, dtype=np.float64) + np.pi / 2.0) * inv2pi  # [F]
    ah = a64.astype(bf16)                                      # single limb
    bh = b64.astype(bf16)
    bl = (b64 - bh.astype(np.float64)).astype(bf16)

    # stationary rows (54): strip0 = [-ah; -bh; -bl; +M; -M; ah(xl dims 0:12)]
    #                       strip1 = [ah; bh; bl; ah(xl dims 12:16)]
    aq = np.zeros((54, FC, 128), dtype=bf16)
    for c in range(FC):
        sl = slice(128 * c, 128 * (c + 1))
        aq[0:16, c, :] = -ah[:, sl]
        aq[16, c, :] = -bh[sl]
        aq[17, c, :] = -bl[sl]
        aq[18, c, :] = bf16(MAGIC)
        aq[19, c, :] = bf16(-MAGIC)
        aq[20:32, c, :] = ah[0:12, sl]
        aq[32:48, c, :] = ah[:, sl]
        aq[48, c, :] = bh[sl]
        aq[49, c, :] = bl[sl]
        aq[50:54, c, :] = ah[12:16, sl]

    scale = math.sqrt(2.0 / F) * W_PRESCALE
    W2 = (np.asarray(W, dtype=np.float64).reshape(F, M) * scale).astype(bf16)
    wsc = np.zeros((128, FC, 2 * M), dtype=bf16)               # zero-padded M
    wsc[:, :, 0:M] = W2.reshape(FC, 128, M).transpose(1, 0, 2)

    sel = np.zeros((112, 16), dtype=bf16)
    for g in range(4):
        for m in range(16):
            sel[32 * g + m, m] = 1.0
    return aq, wsc, sel


def _pack_x(xs):
    """xs [NLOC, D] fp32 -> xq [54, NLOC] bf16 per the row layout above."""
    import ml_dtypes
    bf16 = ml_dtypes.bfloat16
    x64 = np.asarray(xs, dtype=np.float64).T                   # [16, NLOC]
    xh = x64.astype(bf16)
    xl = (x64 - xh.astype(np.float64)).astype(bf16)
    xq = np.zeros((54, NLOC), dtype=bf16)
    xq[0:16] = xh
    xq[16:20] = bf16(1.0)
    xq[20:32] = xl[0:12]
    xq[32:48] = xh
    xq[48:50] = bf16(1.0)
    xq[50:54] = xl[12:16]
    return xq


def make_in_maps(x, a, b, W):
    x = np.ascontiguousarray(np.asarray(x, dtype=np.float32))
    aq, wsc, sel = _host_prep(a, b, W)
    in_maps = []
    for i in range(NCORES):
        in_maps.append({
            "xq_in": _pack_x(x[i * NLOC:(i + 1) * NLOC]),
            "aq_in": aq,
            "wsc_in": wsc,
            "sel_in": sel,
        })
    return in_maps


def kernel(x, a, b, W):
    if "nc" not in _CACHE:
        _CACHE["nc"] = build_nc()
    nc = _CACHE["nc"]
    in_maps = make_in_maps(x, a, b, W)
    res = run_bass_kernel_spmd(nc, in_maps, core_ids=list(range(NCORES)))
    return np.concatenate([r["out"] for r in res.results], axis=0)
